# revision 35
# baseline (speedup 1.0000x reference)
"""Trainium2 Bass kernel for a cross-attention + adaLN-modulated-LN + linear block.

Sharding: 8 cores = 4 batches (B) x 2 token-halves of S=4096.  No collectives.

This version runs the channel-contracting matmuls (q, kv, attn@v, proj) in
fp8e4m3 with MatmulPerfMode.DoubleRow (two contraction rows per PE pass), which
the PE executes at 0.5 cycles/row vs 1.0 for fp32r/bf16.  The score matmuls
(K=D=64, structurally half-width) stay bf16; the LN/final-linear path stays
bf16/fp32r for accuracy.  Numpy simulation of this quantization scheme gives
rel err ~2.5e-3 vs the fp32 reference (gate 2e-2).

Scale bookkeeping: weights are pre-scaled x64 on the host before the fp8 cast
(their std 0.02 sits in e4m3's denormal range otherwise).  q,k are kept scaled
x64 in bf16 and the 64*64 factor folds into the exp scale (SCALE/4096).  The
kv-v and proj PSUM results are evicted with a 1/64 multiply.  bproj is
pre-added into the bf16 residual copy of x on the host; bkv rides as a fifth
ones-slice of the packed v operand.

Device layout is feature-major: activations live as [C, tok] tiles.  The host
pre-transposes and pre-packs everything, so there are no PE transposes for
x/v.  The softmax denominator rides the attn@v matmul as an all-ones lhsT
column; normalization is broadcast with a tiny K=2 selector matmul.  The final
LayerNorm modulation is folded into the output matmul algebraically as in the
baseline.
"""

import sys

for _p in ("/opt/trn_rl_repo", "/opt/pypackages"):
    if _p not in sys.path:
        sys.path.append(_p)

import numpy as np
import ml_dtypes

import concourse.bacc as bacc
import concourse.tile as tile
from concourse import mybir
from concourse.bass_utils import run_bass_kernel_spmd
from concourse.masks import make_identity

FP = mybir.dt.float32
FPR = mybir.dt.float32r
BF = mybir.dt.bfloat16
F8 = mybir.dt.float8e4
AF = mybir.ActivationFunctionType
OP = mybir.AluOpType
DR = mybir.MatmulPerfMode.DoubleRow

NP_F8 = ml_dtypes.float8_e4m3
NP_BF = ml_dtypes.bfloat16


def _r(ap):
    """View an fp32 AP as float32r for full-rate PE matmuls (same bits)."""
    return ap.bitcast(FPR)


# Problem sizes (hardcoded per spec).
B = 4
S = 4096
C = 1024
N2 = 512
H = 16
D = 64
T = 16
NT = 256          # tokens per frame
OUTD = 32

STOK = S // 2     # tokens per core
F = 8             # frames per core
G = C // 128      # 8 channel groups
A = G // 2        # 4 channel-pair groups (DoubleRow)
TB = 512          # token block (matmul N)
NTB = STOK // TB  # 4
KB = N2 // 128    # 4 key blocks
KP = KB // 2      # 2 key-block pairs (DoubleRow)
WS = 64.0         # host-side weight scale for fp8
SCALE = D ** -0.5
EXPSC = SCALE / (WS * WS)
EPS = 1e-6
P = 128
SEG = 192         # vv cols per head-pair segment


# Debug: CoreSim does not implement Silu; sim builds swap it for Sigmoid and
# the debug harness compares against a numpy model with the same swap.
_SILU_FUNC = [AF.Silu]
# Debug: when true, declare extra outputs and dump stage tiles at the end.
_DEBUG_OUTPUTS = [False]


def _body(nc, tc, io):
    with nc.allow_low_precision("fp8 DoubleRow matmuls; bf16 activations"):
        _body_inner(nc, tc, io)


def _body_inner(nc, tc, io):
    xp8, xbf, vp8 = io["xp8"], io["xbf"], io["vp8"]
    tvec, cmat = io["t_b"], io["c_sl"]
    wq8, bq64 = io["wq8"], io["bq64"]
    wkvk8, wv8, bk64 = io["wkvk8"], io["wv8"], io["bk64"]
    wproj8 = io["wproj8"]
    wada_bf, bada = io["wada_bf"], io["bada"]
    wlin, blin = io["wlin"], io["blin"]
    yT = io["yT"]

    with (
        tc.tile_pool(name="consts", bufs=1) as consts,
        tc.tile_pool(name="xbig", bufs=1) as xbigp,
        tc.tile_pool(name="qa", bufs=1) as qap,
        tc.tile_pool(name="kT", bufs=1) as kTp,
        tc.tile_pool(name="wp", bufs=3) as wp,
    ):
        # ---- constants / small inputs ----
        scratch = consts.tile([P, P], FP, tag="scratch")
        make_identity(nc, scratch)
        ident = consts.tile([P, P], FP, tag="ident")
        nc.vector.tensor_copy(out=_r(ident), in_=scratch)
        ones_t = consts.tile([P, P], FP, tag="ones")
        nc.vector.tensor_scalar(
            _r(ones_t), scratch, 0.0, 1.0, op0=OP.mult, op1=OP.add
        )
        ones_bf = consts.tile([P, 2], BF, tag="onesbf")
        nc.vector.memset(ones_bf, 1.0)
        eps_t = consts.tile([P, 1], FP, tag="eps")
        nc.vector.memset(eps_t, EPS)

        bq_t = consts.tile([P, G], FP, tag="bq")
        nc.sync.dma_start(out=bq_t, in_=bq64.ap())
        bk_t = consts.tile([P, G], FP, tag="bk")
        nc.sync.dma_start(out=bk_t, in_=bk64.ap())
        bada_t = consts.tile([P, 16], FP, tag="bada")
        nc.sync.dma_start(out=bada_t, in_=bada.ap())
        blin_row = consts.tile([F, OUTD], FP, tag="blin")
        nc.sync.dma_start(out=blin_row, in_=blin.ap())
        t_t = consts.tile([P, G], FP, tag="tvec")
        nc.sync.dma_start(out=t_t, in_=tvec.ap())
        wlin_sb = consts.tile([P, G, OUTD], FP, tag="wlin")
        nc.sync.dma_start(out=_r(wlin_sb), in_=_r(wlin.ap()))
        silu_t = consts.tile([P, G, F], BF, tag="silu")
        ada_t = consts.tile([P, 16, F], FP, tag="ada")

        # ---- persistent activation buffers ----
        xp8_t = xbigp.tile([P, A, 2, STOK], F8, tag="xp8")    # packed fp8 x^T
        xbf_t = xbigp.tile([P, G, STOK], BF, tag="xbf")       # bf16 x^T + bproj
        x1_t = xbigp.tile([P, G, STOK], BF, tag="x1")         # post-residual
        qa_t = qap.tile([P, G, STOK], BF, tag="qa")           # 64*q (bf16)
        aop8_t = qap.tile([P, A, 2, STOK], F8, tag="aop8")    # packed fp8 attn-out
        kt_t = kTp.tile([P, G, N2], BF, tag="kT")             # 64*k (bf16)
        vp8_t = kTp.tile([P, 5, 2, N2], F8, tag="vp8")        # packed fp8 v^T + ones
        # vv8: key-major fp8 value matrix, packed for DoubleRow over key-block
        # pairs.  [P keys, kp, i(kb within pair), 8 head-pair segs x SEG]:
        # per seg: [0:64] even-head data, [64] ones (denominator), [65:128]
        # junk, [128:192] odd-head data (odd lhsT window starts at col 64).
        vv8_t = kTp.tile([P, KP, 2, G * SEG], F8, tag="vv8")
        nc.vector.memset(vv8_t, 0.0)
        nc.vector.memset(
            vv8_t.rearrange("p k i (g s) -> p k i g s", s=SEG)[:, :, :, :, 64:65],
            1.0,
        )

        # ---- big input DMAs (overlap with early compute) ----
        wproj_t = qap.tile([P, G, A, 2, P], F8, tag="wproj")
        nc.scalar.dma_start(out=wproj_t, in_=wproj8.ap())
        nc.scalar.dma_start(out=xp8_t, in_=xp8.ap())
        nc.gpsimd.dma_start(
            out=xbf_t[:, 0:4, :], in_=xbf.ap()[:, 0:4, :]
        )
        nc.gpsimd.dma_start(
            out=xbf_t[:, 4:8, :], in_=xbf.ap()[:, 4:8, :]
        )
        nc.sync.dma_start(out=vp8_t, in_=vp8.ap())

        with tc.tile_pool(name="psA", bufs=8, space="PSUM") as psA:
            # ---- kv projection (fp8 DoubleRow) ----
            # k part: cout-major [couts, keys], evict to kt_t (bf16, x64).
            for g in range(G):
                wk_t = wp.tile([P, A, 2, P], F8, name="wk", tag="w")
                nc.sync.dma_start(out=wk_t, in_=wkvk8.ap()[g])
                psk = psA.tile([P, N2], FP, name="psk", tag="ps")
                for a in range(A):
                    nc.tensor.matmul(
                        psk,
                        lhsT=wk_t[:, a, :, :],
                        rhs=vp8_t[:, a, :, :],
                        start=(a == 0),
                        stop=(a == A - 1),
                        perf_mode=DR,
                    )
                nc.vector.tensor_scalar_add(kt_t[:, g, :], psk, bk_t[:, g : g + 1])

            # v part: key-major [keys, couts]; bias rides slice a=4 of vp8/wv8.
            for half in range(2):
                wv_t = wp.tile([P, 5, 2, TB], F8, name="wv", tag="w")
                nc.scalar.dma_start(out=wv_t, in_=wv8.ap()[half])
                for kb in range(KB):
                    psv = psA.tile([P, TB], FP, name="psv", tag="ps")
                    for a in range(5):
                        nc.tensor.matmul(
                            psv,
                            lhsT=vp8_t[:, a, :, kb * P : (kb + 1) * P],
                            rhs=wv_t[:, a, :, :],
                            start=(a == 0),
                            stop=(a == 4),
                            perf_mode=DR,
                        )
                    # evict into packed vv8 segments with 1/64 descale
                    vvr = vv8_t.rearrange("p k i (g s) -> p k i g s", s=SEG)
                    out_ap = vvr[:, kb // 2, kb % 2, half * 4 : half * 4 + 4, :]
                    src = psv.rearrange("p (a q j) -> p a q j", q=2, j=64)
                    nc.vector.tensor_scalar_mul(
                        out_ap[:, :, 0:64], src[:, :, 0, :], 1.0 / WS
                    )
                    nc.vector.tensor_scalar_mul(
                        out_ap[:, :, 128:192], src[:, :, 1, :], 1.0 / WS
                    )

            # ---- adaLN: silu(t + c) @ wada + bada (bf16) ----
            c_nat = wp.tile([F, C], FP, name="cnat", tag="misc", bufs=1)
            nc.sync.dma_start(out=_r(c_nat), in_=_r(cmat[:, :]))
            for g in range(G):
                pt = psA.tile([P, F], FP, name="ptc", tag="ps")
                nc.tensor.transpose(
                    _r(pt), _r(c_nat[:, g * P : (g + 1) * P]), _r(ident[0:F, 0:F])
                )
                nc.scalar.activation(
                    out=silu_t[:, g, :],
                    in_=pt,
                    func=_SILU_FUNC[0],
                    bias=t_t[:, g : g + 1],
                    scale=1.0,
                )
            for ct in range(16):
                wa_t = wp.tile([P, G, P], BF, name="wa", tag="w")
                nc.sync.dma_start(out=wa_t, in_=wada_bf.ap()[ct])
                pa = psA.tile([P, F], FP, name="pta", tag="ps")
                for ci in range(G):
                    nc.tensor.matmul(
                        pa,
                        lhsT=wa_t[:, ci, :],
                        rhs=silu_t[:, ci, :],
                        start=(ci == 0),
                        stop=(ci == G - 1),
                    )
                if ct < 8:
                    # ada cols 0..1023 = shift -> ct 0..7
                    nc.vector.tensor_scalar_add(
                        _r(ada_t[:, ct, :]), pa, bada_t[:, ct : ct + 1]
                    )
                else:
                    # ada cols 1024..2047 = sc -> ct 8..15 hold (1 + sc)
                    nc.vector.tensor_scalar(
                        _r(ada_t[:, ct, :]),
                        pa,
                        bada_t[:, ct : ct + 1],
                        1.0,
                        op0=OP.add,
                        op1=OP.add,
                    )

            # ---- q projection (fp8 DoubleRow), evict 64*q to bf16 ----
            for g in range(G):
                wq_t = wp.tile([P, A, 2, P], F8, name="wqt", tag="w")
                nc.sync.dma_start(out=wq_t, in_=wq8.ap()[g])
                for tb in range(NTB):
                    psq = psA.tile([P, TB], FP, name="psq", tag="ps")
                    for a in range(A):
                        nc.tensor.matmul(
                            psq,
                            lhsT=wq_t[:, a, :, :],
                            rhs=xp8_t[:, a, :, tb * TB : (tb + 1) * TB],
                            start=(a == 0),
                            stop=(a == A - 1),
                            perf_mode=DR,
                        )
                    nc.scalar.activation(
                        out=qa_t[:, g, tb * TB : (tb + 1) * TB],
                        in_=psq,
                        func=AF.Identity,
                        bias=bq_t[:, g : g + 1],
                        scale=1.0,
                    )

        # ---- attention (per head; pipelined over (tb, g)) ----
        psC_cm = tc.tile_pool(name="psC", bufs=2, space="PSUM")
        psC = psC_cm.__enter__()
        with (
            tc.tile_pool(name="sc", bufs=3, space="PSUM") as scp,
            tc.tile_pool(name="ao", bufs=2, space="PSUM") as aop,
            tc.tile_pool(name="bc", bufs=1, space="PSUM") as bcpp,
            tc.tile_pool(name="exp", bufs=4) as expp,
            tc.tile_pool(name="dn", bufs=3) as dnp,
        ):
            for tb in range(NTB):
                tbs = slice(tb * TB, (tb + 1) * TB)
                for g in range(G):
                    a_, i_ = g // 2, g % 2
                    dnb = dnp.tile([P, TB], FP, tag="dn")
                    for half in range(2):
                        r0 = half * 64
                        dr_row = 64 - 64 * half  # denom row: 64 (even), 0 (odd)
                        if half == 0:
                            lhs_lo, lhs_hi = g * SEG, g * SEG + 65
                            ao_ps = aop.tile([65, TB], FP, name="aoe", tag="ao")
                            ao_rows = slice(0, 64)
                        else:
                            lhs_lo, lhs_hi = g * SEG + 64, g * SEG + SEG
                            ao_ps = aop.tile([P, TB], FP, name="aoo", tag="ao")
                            ao_rows = slice(64, P)
                        for kp in range(KP):
                            ex = expp.tile([P, 2, TB], F8, tag="e")
                            for i in range(2):
                                kb = 2 * kp + i
                                sc_ps = scp.tile([P, TB], FP, name="scs", tag="sc")
                                nc.tensor.matmul(
                                    sc_ps,
                                    lhsT=kt_t[r0 : r0 + 64, g, kb * P : (kb + 1) * P],
                                    rhs=qa_t[r0 : r0 + 64, g, tbs],
                                    start=True,
                                    stop=True,
                                )
                                nc.scalar.activation(
                                    out=ex[:, i, :], in_=sc_ps, func=AF.Exp,
                                    scale=EXPSC,
                                )
                            nc.tensor.matmul(
                                ao_ps,
                                lhsT=vv8_t[:, kp, :, lhs_lo:lhs_hi],
                                rhs=ex,
                                start=(kp == 0),
                                stop=(kp == KP - 1),
                                perf_mode=DR,
                            )
                        # reciprocal of the ones-column denom row, rounded to
                        # fp32r so the broadcast matmul may consume it
                        # (reciprocal_approx_fast produces garbage on this HW
                        # path -- custom-DVE tables appear not to load)
                        nc.vector.reciprocal(
                            out=_r(dnb[dr_row : dr_row + 1, :]),
                            in_=ao_ps[dr_row : dr_row + 1, :],
                        )
                        # broadcast 1/d to all rows via PE, copy to SBUF,
                        # then normalize + quantize to fp8
                        bc_ps = bcpp.tile([P, TB], FP, name="bcp", tag="bc")
                        nc.tensor.matmul(
                            bc_ps,
                            lhsT=_r(ones_t[dr_row : dr_row + 1, :]),
                            rhs=_r(dnb[dr_row : dr_row + 1, :]),
                            start=True,
                            stop=True,
                        )
                        bcs = dnp.tile([P, TB], FP, tag="bcs")
                        nc.vector.tensor_copy(
                            out=bcs[r0 : r0 + 64, :], in_=bc_ps[r0 : r0 + 64, :]
                        )
                        nc.vector.tensor_tensor(
                            aop8_t[r0 : r0 + 64, a_, i_, tbs],
                            ao_ps[ao_rows, :],
                            bcs[r0 : r0 + 64, :],
                            OP.mult,
                        )

        # ---- proj (fp8 DoubleRow) + residual -> x1 (bf16) ----
        # tb-outer so proj for tb overlaps attention of tb+1; weights preloaded
        for tb in range(NTB):
            tbs = slice(tb * TB, (tb + 1) * TB)
            for g in range(G):
                pst = psC.tile([P, TB], FP, name="psp", tag="ps")
                for a in range(A):
                    nc.tensor.matmul(
                        pst,
                        lhsT=wproj_t[:, g, a, :, :],
                        rhs=aop8_t[:, a, :, tbs],
                        start=(a == 0),
                        stop=(a == A - 1),
                        perf_mode=DR,
                    )
                # x1 = pst/64 + (x + bproj)   (bproj pre-added on host)
                nc.vector.scalar_tensor_tensor(
                    out=x1_t[:, g, tbs],
                    in0=pst,
                    scalar=1.0 / WS,
                    in1=xbf_t[:, g, tbs],
                    op0=OP.mult,
                    op1=OP.add,
                )

        psC_cm.__exit__(None, None, None)

        # ---- LN + folded modulation + final linear ----
        with (
            tc.tile_pool(name="psD", bufs=6, space="PSUM") as psD,
            tc.tile_pool(name="tmp", bufs=3) as tmpp,
            tc.tile_pool(name="st", bufs=8) as stp,
            tc.tile_pool(name="w1", bufs=3) as w1p,
            tc.tile_pool(name="rows", bufs=4) as rowp,
            tc.tile_pool(name="nrm", bufs=2) as nrmp,
            tc.tile_pool(name="yo", bufs=2) as yop,
        ):
            # bf16 copies of ada and wlin for the cheap per-frame row matmuls
            ada_bf = rowp.tile([P, 16, F], BF, name="adabf", tag="adabf", bufs=1)
            nc.vector.tensor_copy(out=ada_bf, in_=ada_t)
            wlin_bf = rowp.tile([P, G, OUTD], BF, name="wlinbf", tag="wlinbf", bufs=1)
            nc.vector.tensor_copy(out=wlin_bf, in_=wlin_sb)

            for tb in range(NTB):
                tbs = slice(tb * TB, (tb + 1) * TB)
                ln_a = psD.tile([1, TB], FP, name="lna", tag="ps")
                ln_b = psD.tile([1, TB], FP, name="lnb", tag="ps")
                for g in range(G):
                    sqt = tmpp.tile([P, TB], BF, tag="tmp")
                    nc.gpsimd.tensor_tensor(
                        sqt, x1_t[:, g, tbs], x1_t[:, g, tbs], OP.mult
                    )
                    nc.tensor.matmul(
                        ln_a,
                        lhsT=ones_bf[:, 0:1],
                        rhs=x1_t[:, g, tbs],
                        start=(g == 0),
                        stop=(g == G - 1),
                    )
                    nc.tensor.matmul(
                        ln_b,
                        lhsT=ones_bf[:, 0:1],
                        rhs=sqt,
                        start=(g == 0),
                        stop=(g == G - 1),
                    )
                mu = stp.tile([1, TB], FP, name="mu", tag="st")
                std = stp.tile([1, TB], FP, name="std", tag="st")
                rst = stp.tile([1, TB], FP, name="rst", tag="st")
                mu_bf = stp.tile([1, TB], BF, name="mub", tag="st")
                std_bf = stp.tile([1, TB], BF, name="stdb", tag="st")
                nc.vector.tensor_scalar_mul(_r(mu), ln_a, 1.0 / C)
                nc.vector.tensor_mul(_r(std), mu, mu)
                nc.vector.scalar_tensor_tensor(
                    out=_r(std),
                    in0=ln_b,
                    scalar=1.0 / C,
                    in1=std,
                    op0=OP.mult,
                    op1=OP.subtract,
                )
                nc.scalar.activation(
                    out=_r(std), in_=std, func=AF.Sqrt, bias=eps_t[0:1, :], scale=1.0
                )
                nc.vector.reciprocal(_r(rst), std)
                nc.gpsimd.tensor_copy(out=mu_bf, in_=mu)
                nc.gpsimd.tensor_copy(out=std_bf, in_=std)
                bc32_ps = psD.tile([32, TB], FP, name="bc32", tag="ps")
                nc.tensor.matmul(
                    bc32_ps,
                    lhsT=_r(ones_t[0:1, 0:32]),
                    rhs=_r(rst),
                    start=True,
                    stop=True,
                )
                bc32 = nrmp.tile([32, TB], FP, tag="nrm")
                nc.scalar.copy(out=bc32, in_=bc32_ps)
                for f2 in range(2):
                    f = tb * 2 + f2
                    fcs = slice(f2 * NT, (f2 + 1) * NT)  # cols within tb
                    gcs = slice(tb * TB + f2 * NT, tb * TB + (f2 + 1) * NT)
                    w1 = w1p.tile([P, G, OUTD], BF, tag="w1")
                    for g in range(G):
                        nc.gpsimd.tensor_scalar_mul(
                            w1[:, g, :],
                            wlin_sb[:, g, :],
                            ada_t[:, 8 + g, f : f + 1],
                        )
                    ws1_ps = psD.tile([1, OUTD], FP, name="ws1", tag="ps")
                    c2_ps = psD.tile([1, OUTD], FP, name="c2", tag="ps")
                    for g in range(G):
                        nc.tensor.matmul(
                            ws1_ps,
                            lhsT=ada_bf[:, 8 + g, f : f + 1],
                            rhs=wlin_bf[:, g, :],
                            start=(g == 0),
                            stop=(g == G - 1),
                        )
                        nc.tensor.matmul(
                            c2_ps,
                            lhsT=ada_bf[:, g, f : f + 1],
                            rhs=wlin_bf[:, g, :],
                            start=(g == 0),
                            stop=(g == G - 1),
                        )
                    ws1n = rowp.tile([1, OUTD], BF, name="ws1n", tag="rows")
                    c2b = rowp.tile([1, OUTD], BF, name="c2b", tag="rows")
                    nc.vector.tensor_scalar_mul(ws1n, ws1_ps, -1.0)
                    nc.vector.tensor_tensor(c2b, c2_ps, blin_row[0:1, :], OP.add)
                    y_ps = psD.tile([OUTD, NT], FP, name="yps", tag="ps")
                    for g in range(G):
                        nc.tensor.matmul(
                            y_ps,
                            lhsT=w1[:, g, :],
                            rhs=x1_t[:, g, gcs],
                            start=(g == 0),
                            stop=False,
                        )
                    nc.tensor.matmul(
                        y_ps,
                        lhsT=ws1n,
                        rhs=mu_bf[0:1, fcs],
                        start=False,
                        stop=False,
                    )
                    nc.tensor.matmul(
                        y_ps,
                        lhsT=c2b,
                        rhs=std_bf[0:1, fcs],
                        start=False,
                        stop=True,
                    )
                    yt = yop.tile([OUTD, NT], FP, tag="y")
                    nc.vector.tensor_mul(yt, y_ps, bc32[:, fcs])
                    nc.sync.dma_start(out=yT[:, gcs], in_=yt)

        if _DEBUG_OUTPUTS[0]:
            nc.sync.dma_start(out=io["dbg_qa"].ap(), in_=qa_t)
            nc.sync.dma_start(out=io["dbg_kt"].ap(), in_=kt_t)
            nc.sync.dma_start(out=io["dbg_vv8"].ap(), in_=vv8_t)
            nc.sync.dma_start(out=io["dbg_aop8"].ap(), in_=aop8_t)
            nc.sync.dma_start(out=io["dbg_x1"].ap(), in_=x1_t)
            nc.sync.dma_start(out=io["dbg_ada"].ap(), in_=ada_t)
            nc.sync.dma_start(out=io["dbg_xp8"].ap(), in_=xp8_t)


def declare_io(nc):
    return {
        "xp8": nc.dram_tensor("xp8", [P, A, 2, STOK], F8, kind="ExternalInput"),
        "xbf": nc.dram_tensor("xbf", [P, G, STOK], BF, kind="ExternalInput"),
        "vp8": nc.dram_tensor("vp8", [P, 5, 2, N2], F8, kind="ExternalInput"),
        "t_b": nc.dram_tensor("t_b", [P, G], FP, kind="ExternalInput"),
        "c_sl": nc.dram_tensor("c_sl", [F, C], FP, kind="ExternalInput"),
        "wq8": nc.dram_tensor("wq8", [G, P, A, 2, P], F8, kind="ExternalInput"),
        "bq64": nc.dram_tensor("bq64", [P, G], FP, kind="ExternalInput"),
        "wkvk8": nc.dram_tensor("wkvk8", [G, P, A, 2, P], F8, kind="ExternalInput"),
        "wv8": nc.dram_tensor("wv8", [2, P, 5, 2, TB], F8, kind="ExternalInput"),
        "bk64": nc.dram_tensor("bk64", [P, G], FP, kind="ExternalInput"),
        "wproj8": nc.dram_tensor("wproj8", [P, G, A, 2, P], F8, kind="ExternalInput"),
        "wada_bf": nc.dram_tensor("wada_bf", [16, P, G, P], BF, kind="ExternalInput"),
        "bada": nc.dram_tensor("bada", [P, 16], FP, kind="ExternalInput"),
        "wlin": nc.dram_tensor("wlin", [P, G, OUTD], FP, kind="ExternalInput"),
        "blin": nc.dram_tensor("blin", [F, OUTD], FP, kind="ExternalInput"),
        "yT": nc.dram_tensor("yT", [OUTD, STOK], FP, kind="ExternalOutput"),
        **(
            {
                "dbg_qa": nc.dram_tensor("dbg_qa", [P, G, STOK], BF, kind="ExternalOutput"),
                "dbg_kt": nc.dram_tensor("dbg_kt", [P, G, N2], BF, kind="ExternalOutput"),
                "dbg_vv8": nc.dram_tensor("dbg_vv8", [P, KP, 2, G * SEG], F8, kind="ExternalOutput"),
                "dbg_aop8": nc.dram_tensor("dbg_aop8", [P, A, 2, STOK], F8, kind="ExternalOutput"),
                "dbg_x1": nc.dram_tensor("dbg_x1", [P, G, STOK], BF, kind="ExternalOutput"),
                "dbg_ada": nc.dram_tensor("dbg_ada", [P, 16, F], FP, kind="ExternalOutput"),
                "dbg_xp8": nc.dram_tensor("dbg_xp8", [P, A, 2, STOK], F8, kind="ExternalOutput"),
            }
            if _DEBUG_OUTPUTS[0]
            else {}
        ),
    }


def build_nc():
    nc = bacc.Bacc("TRN2", target_bir_lowering=False, debug=False)
    io = declare_io(nc)
    with tile.TileContext(nc) as tc:
        _body(nc, tc, io)
    nc.compile()
    return nc


_CACHE = {}


def _get_nc():
    if "nc" not in _CACHE:
        _CACHE["nc"] = build_nc()
    return _CACHE["nc"]


def _pack_cin_pairs(w):
    """[C_in, M] -> [P, A, 2, M] fp8 with x64 scale (DoubleRow lhsT layout)."""
    m = w.shape[1]
    blk = (w * WS).reshape(A, 2, P, m).transpose(2, 0, 1, 3)
    return np.ascontiguousarray(blk).astype(NP_F8)


def make_in_maps(x, v, t, c, wq, bq, wkv, bkv, wproj, bproj, wada, bada, wlin, blin):
    f32 = lambda a: np.asarray(a, dtype=np.float32)
    x, v, t, c = f32(x), f32(v), f32(t), f32(c)
    wq, wkv, wproj, wada = f32(wq), f32(wkv), f32(wproj), f32(wada)
    bq, bkv, bproj, bada_v = f32(bq), f32(bkv), f32(bproj), f32(bada)
    wlin, blin = f32(wlin), f32(blin)

    def pg(vec):  # [C] -> [P, G] with c = g*128+p
        return np.ascontiguousarray(vec.reshape(G, P).T)

    wq8_h = np.stack([_pack_cin_pairs(wq[:, g * P : (g + 1) * P]) for g in range(G)])
    wkvk8_h = np.stack(
        [_pack_cin_pairs(wkv[:, g * P : (g + 1) * P]) for g in range(G)]
    )
    wproj8_h = np.ascontiguousarray(
        np.stack(
            [_pack_cin_pairs(wproj[:, g * P : (g + 1) * P]) for g in range(G)]
        ).transpose(1, 0, 2, 3, 4)
    )
    # wv8: [2, P, 5, 2, TB]; slice a=4 row (p=0,i=0) carries 64*bkvv
    wv8_h = np.zeros((2, P, 5, 2, TB), NP_F8)
    for half in range(2):
        wslab = _pack_cin_pairs(wkv[:, C + half * TB : C + (half + 1) * TB])
        wv8_h[half, :, 0:4] = wslab
        wv8_h[half, 0, 4, 0, :] = (WS * bkv[C + half * TB : C + (half + 1) * TB]).astype(
            NP_F8
        )
    wada_bf_h = np.ascontiguousarray(
        wada.reshape(G, P, 16, P).transpose(2, 1, 0, 3)
    ).astype(NP_BF)
    wlin_h = np.ascontiguousarray(wlin.reshape(G, P, OUTD).transpose(1, 0, 2))

    shared = {
        "wq8": wq8_h,
        "bq64": pg(WS * bq),
        "wkvk8": wkvk8_h,
        "wv8": wv8_h,
        "bk64": pg(WS * bkv[:C]),
        "wproj8": wproj8_h,
        "wada_bf": wada_bf_h,
        "bada": np.ascontiguousarray(bada_v.reshape(16, P).T),
        "wlin": wlin_h,
        "blin": np.ascontiguousarray(np.tile(blin.reshape(1, OUTD), (F, 1))),
    }
    in_maps = []
    for m in range(8):
        b, half = divmod(m, 2)
        xs = x[b, half * STOK : (half + 1) * STOK, :]          # [STOK, C]
        xT = np.ascontiguousarray(xs.T)                        # [C, STOK]
        xp8_h = np.ascontiguousarray(
            xT.reshape(A, 2, P, STOK).transpose(2, 0, 1, 3)
        ).astype(NP_F8)
        xbf_h = np.ascontiguousarray(
            (xT + bproj[:, None]).reshape(G, P, STOK).transpose(1, 0, 2)
        ).astype(NP_BF)
        vT = np.ascontiguousarray(v[b].T)                      # [C, N2]
        vp8_h = np.zeros((P, 5, 2, N2), NP_F8)
        vp8_h[:, 0:4] = vT.reshape(A, 2, P, N2).transpose(2, 0, 1, 3).astype(NP_F8)
        vp8_h[0, 4, 0, :] = np.float32(1.0).astype(NP_F8)
        in_maps.append(
            {
                "xp8": xp8_h,
                "xbf": xbf_h,
                "vp8": vp8_h,
                "t_b": pg(t[b]),
                "c_sl": np.ascontiguousarray(c[b, half * F : (half + 1) * F, :]),
                **shared,
            }
        )
    return in_maps


def assemble_y(results):
    y = np.empty((B, T, NT, OUTD), np.float32)
    for m in range(8):
        b, half = divmod(m, 2)
        yt = np.asarray(results[m]["yT"])  # [OUTD, STOK]
        y[b, half * F : (half + 1) * F] = yt.T.reshape(F, NT, OUTD)
    return y


def kernel(x, v, t, c, wq, bq, wkv, bkv, wproj, bproj, wada, bada, wlin, blin, T=16, H=16):
    nc = _get_nc()
    in_maps = make_in_maps(
        x, v, t, c, wq, bq, wkv, bkv, wproj, bproj, wada, bada, wlin, blin
    )
    res = run_bass_kernel_spmd(nc, in_maps, core_ids=list(range(8)))
    return assemble_y(res.results)
